# revision 11
# baseline (speedup 1.0000x reference)
"""RBF kernel ridge regression inference on 8 Trainium2 NeuronCores — v3.

out[q] = sum_t exp(-g*||X[q]-T[t]||^2) * coef[t]
       = sum_t exp(2g*dot[q,t] - g*x2[q]) * w[t],   w[t] = coef[t]*exp(-g*y2[t])

Layout: queries on PSUM partitions, train on the free dim.  The GEMM runs in
fp8e4 DoubleRow (2 MACs/PE/cycle) with the query operand stationary; matmuls
are ordered dc-outer so each stationary weight is reused across the 4 psum
sections (2 LDWEIGHTS per tile instead of 8).  ScalarE evaluates one fused exp
per [128 x 2048] PSUM span with per-partition bias -g*x2[q] (precomputed on
the host, like w) and scale 2g.  The coef-weighted reduction over train runs
as a single fused tensor_tensor_reduce per tile on DVE (custom uop:
out = et*w, accum_out = sum), which avoids the slower scalar_tensor_tensor
path.  Queries are sharded across the 8 cores; train_X and w are replicated.
"""

import numpy as np
import ml_dtypes

GAMMA = 1.0
N_QUERY, N_TRAIN, D = 8192, 8192, 512
N_CORES = 8
P = 128
QPC = N_QUERY // N_CORES   # 1024 queries per core
NQP = QPC // P             # 8 query chunks (psum partition dim)
NDC = 2                    # DoubleRow contraction chunks (K=256 each)
TSEC = 512                 # train cols per matmul (one psum bank)
TGRP = 2048                # train cols per psum tile (4 banks)
NTG = N_TRAIN // TGRP      # 4 tile groups per query chunk
NSEC = TGRP // TSEC        # 4 sections per tile
N_WARMUP = 7               # HAM warmup matmuls (~3.4us of PE activity needed)
NPAIR_BUF = 3              # et pair buffers (each holds 2 tiles = 4096 cols)

_CACHE = {}


def _build_program():
    from contextlib import ExitStack

    import concourse.bass as bass
    import concourse.mybir as mybir
    import concourse.tile as tile
    from concourse import bacc

    f32 = mybir.dt.float32
    bf16 = mybir.dt.bfloat16
    fp8 = mybir.dt.float8e4
    AF = mybir.ActivationFunctionType
    MUL = mybir.AluOpType.mult
    ADD = mybir.AluOpType.add
    DR = mybir.MatmulPerfMode.DoubleRow

    nc = bacc.Bacc(
        "TRN2", target_bir_lowering=False, debug=False, num_devices=N_CORES
    )

    # host layout: [p, dc, j, q/t] with d = dc*256 + j*128 + p
    xt_d = nc.dram_tensor("xt_fp8", [P, NDC * 2 * QPC], fp8, kind="ExternalInput").ap()
    tt_d = nc.dram_tensor("tt_fp8", [P, NDC * 2 * N_TRAIN], fp8, kind="ExternalInput").ap()
    w_d = nc.dram_tensor("w_bf16", [P, N_TRAIN], bf16, kind="ExternalInput").ap()
    x2_d = nc.dram_tensor("x2_f32", [P, NQP], f32, kind="ExternalInput").ap()
    out_d = nc.dram_tensor("out", [QPC], f32, kind="ExternalOutput").ap()

    with tile.TileContext(nc) as tc, ExitStack() as ctx:
        res = ctx.enter_context(tc.tile_pool(name="res", bufs=1))
        scrpool = ctx.enter_context(tc.tile_pool(name="scr", bufs=2))
        pspool = ctx.enter_context(tc.tile_pool(name="ps", bufs=2, space="PSUM"))

        # warm the exp table while input DMAs run
        warm = res.tile([1, 1], f32, tag="warm", name="warm")
        nc.vector.memset(warm[:], 0.0)
        nc.scalar.activation(warm[:], warm[:], AF.Exp)

        # warm the PE clock (HAM lifts the gate to 2.4GHz after ~3.4us of
        # sustained matmul activity) with dummy DR matmuls on a zeroed tile
        wlhs = res.tile([P, 2, P], fp8, tag="wlhs", name="wlhs")
        wrhs = res.tile([P, 2, TSEC], fp8, tag="wrhs", name="wrhs")
        nc.vector.memset(wlhs[:], 0.0)
        nc.vector.memset(wrhs[:], 0.0)
        wps = pspool.tile([P, TGRP], f32, tag="ps", name="wps")
        for i in range(N_WARMUP):
            nc.tensor.matmul(
                wps[:, :TSEC], wlhs[:], wrhs[:], start=True, stop=True,
                perf_mode=DR,
            )

        # ---- DMAs ordered by first use ----
        x2n = res.tile([P, NQP], f32, tag="x2n")
        xt_sb = res.tile([P, NDC, 2, QPC], fp8, tag="xt")
        w_sb = res.tile([P, N_TRAIN], bf16, tag="w")
        tt_sb = [
            res.tile([P, 2, N_TRAIN], fp8, tag=f"tt{dc}", name=f"tt{dc}")
            for dc in range(NDC)
        ]
        tt_view = tt_d.rearrange("p (dc j t) -> p dc j t", dc=NDC, j=2)
        nc.sync.dma_start(x2n[:], x2_d)
        nc.sync.dma_start(
            xt_sb[:], xt_d.rearrange("p (dc j q) -> p dc j q", dc=NDC, j=2)
        )
        for tg in range(NTG):
            t0, t1 = tg * TGRP, (tg + 1) * TGRP
            for dc in range(NDC):
                nc.sync.dma_start(tt_sb[dc][:, :, t0:t1], tt_view[:, dc, :, t0:t1])
            nc.sync.dma_start(w_sb[:, t0:t1], w_d[:, t0:t1])

        # ---- main loop: tg-pair major; each (qp, half) builds a 4096-col et
        # pair buffer from two psum tiles, then one fused STT reduces it (the
        # wider STT amortizes the per-instruction DVE overhead) ----
        NHALF = NTG // 2
        Spart = res.tile([P, NQP * NHALF], f32, tag="Spart")
        pair_bufs = [
            res.tile([P, 2 * TGRP], bf16, tag=f"pb{i}", name=f"pb{i}")
            for i in range(NPAIR_BUF)
        ]
        pair_idx = 0
        for half in range(NHALF):
            for qp in range(NQP):
                pb = pair_bufs[pair_idx % NPAIR_BUF]
                pair_idx += 1
                for sub in range(2):
                    tg = 2 * half + sub
                    ps = pspool.tile([P, TGRP], f32, tag="ps")
                    for dc in range(NDC):
                        for sec in range(NSEC):
                            t0 = tg * TGRP + sec * TSEC
                            nc.tensor.matmul(
                                ps[:, sec * TSEC : (sec + 1) * TSEC],
                                xt_sb[:, dc, :, qp * P : (qp + 1) * P],
                                tt_sb[dc][:, :, t0 : t0 + TSEC],
                                start=(dc == 0),
                                stop=(dc == NDC - 1),
                                perf_mode=DR,
                            )
                    nc.scalar.activation(
                        pb[:, sub * TGRP : (sub + 1) * TGRP], ps[:], AF.Exp,
                        bias=x2n[:, qp : qp + 1], scale=2.0 * GAMMA,
                    )
                idx = qp * NHALF + half
                scr = scrpool.tile([P, 2 * TGRP], bf16, tag="rscr")
                nc.vector.scalar_tensor_tensor(
                    scr[:], pb[:], 1.0,
                    w_sb[:, half * 2 * TGRP : (half + 1) * 2 * TGRP],
                    MUL, MUL, accum_out=Spart[:, idx : idx + 1],
                )

        # ---- epilogue: out[qp] = sum over half slots per query ----
        # out dram holds out[p*NQP + qp] = result for query qp*128+p: each
        # partition writes one contiguous 32B run (fast DMA); the host
        # transposes back (pure unshard gather).
        outcol = res.tile([P, NQP], f32, tag="outcol")
        for qp in range(NQP):
            nc.vector.tensor_reduce(
                outcol[:, qp : qp + 1],
                Spart[:, qp * NHALF : (qp + 1) * NHALF],
                axis=mybir.AxisListType.X,
                op=ADD,
            )
        nc.sync.dma_start(out_d.rearrange("(p c) -> p c", c=NQP), outcol[:])

    nc.compile()
    return nc


def _get_program():
    if "nc" not in _CACHE:
        _CACHE["nc"] = _build_program()
    return _CACHE["nc"]


def _interleave_fp8(mat_t):
    """[D, N] f32 -> [P, NDC*2*N] fp8 with host layout (p, dc, j, n),
    d = dc*256 + j*128 + p."""
    D_, N = mat_t.shape
    v = mat_t.reshape(NDC, 2, P, N)            # [dc, j, p, n]
    v = np.transpose(v, (2, 0, 1, 3))          # [p, dc, j, n]
    return np.ascontiguousarray(v.reshape(P, NDC * 2 * N)).astype(
        ml_dtypes.float8_e4m3
    )


def make_in_maps(X, train_X, dual_coef):
    bf = ml_dtypes.bfloat16
    # w[t] = coef[t] * exp(-g*||T[t]||^2): model-side constant, replicated.
    y2 = np.sum(train_X.astype(np.float64) ** 2, axis=1)
    w = (dual_coef.astype(np.float64) * np.exp(-GAMMA * y2)).astype(np.float32)
    ttb = _interleave_fp8(np.ascontiguousarray(train_X.T))
    wrep = np.ascontiguousarray(np.broadcast_to(w[None, :], (P, N_TRAIN))).astype(bf)
    # query-side bias -g*||X[q]||^2, laid out [p, qp] per core
    x2 = -GAMMA * np.sum(X.astype(np.float64) ** 2, axis=1)
    XT = np.ascontiguousarray(X.T)
    in_maps = []
    for c in range(N_CORES):
        xs = XT[:, c * QPC : (c + 1) * QPC]
        x2c = x2[c * QPC : (c + 1) * QPC].reshape(NQP, P).T  # [p, qp]
        in_maps.append(
            {
                "xt_fp8": _interleave_fp8(np.ascontiguousarray(xs)),
                "tt_fp8": ttb,
                "w_bf16": wrep,
                "x2_f32": np.ascontiguousarray(x2c).astype(np.float32),
            }
        )
    return in_maps


def _get_callable():
    """Cached jax shard_map callable for the 8-core NEFF execution."""
    if "call" in _CACHE:
        return _CACHE["call"]

    import jax
    from jax.sharding import Mesh, PartitionSpec
    from jax.experimental.shard_map import shard_map

    import concourse.mybir as mybir
    from concourse import bass2jax
    from concourse.bass2jax import install_neuronx_cc_hook

    install_neuronx_cc_hook()
    nc = _get_program()

    partition_name = (
        nc.partition_id_tensor.name if nc.partition_id_tensor else None
    )
    in_names, out_names, out_avals, zero_outs = [], [], [], []
    for alloc in nc.m.functions[0].allocations:
        if not isinstance(alloc, mybir.MemoryLocationSet):
            continue
        if alloc.kind not in ("ExternalInput", "ExternalOutput"):
            continue
        name = alloc.memorylocations[0].name
        if alloc.kind == "ExternalInput":
            if name != partition_name:
                in_names.append(name)
        else:
            out_names.append(name)
            shape = tuple(alloc.tensor_shape)
            dtype = mybir.dt.np(alloc.dtype)
            out_avals.append(jax.core.ShapedArray(shape, dtype))
            zero_outs.append(np.zeros(shape, dtype))
    all_in_names = in_names + out_names
    if partition_name is not None:
        all_in_names = all_in_names + [partition_name]

    def _body(*args):
        operands = list(args)
        if partition_name is not None:
            operands.append(bass2jax.partition_id_tensor())
        outs = bass2jax._bass_exec_p.bind(
            *operands,
            out_avals=tuple(out_avals),
            in_names=tuple(all_in_names),
            out_names=tuple(out_names),
            lowering_input_output_aliases=(),
            sim_require_finite=True,
            sim_require_nnan=True,
            nc=nc,
        )
        return tuple(outs)

    devices = jax.devices()[:N_CORES]
    mesh = Mesh(np.asarray(devices), ("core",))
    n_all = len(in_names) + len(out_names)
    fn = jax.jit(
        shard_map(
            _body,
            mesh=mesh,
            in_specs=(PartitionSpec("core"),) * n_all,
            out_specs=(PartitionSpec("core"),) * len(out_names),
            check_rep=False,
        ),
        keep_unused=True,
    )
    _CACHE["call"] = (fn, in_names, out_names, out_avals, zero_outs, mesh)
    return _CACHE["call"]


def concat_inputs(in_maps):
    fn, in_names, out_names, out_avals, zero_outs, mesh = _get_callable()
    concat_in = [
        np.concatenate([np.asarray(m[name]) for m in in_maps], axis=0)
        for name in in_names
    ]
    concat_zeros = [
        np.zeros((N_CORES * z.shape[0], *z.shape[1:]), z.dtype) for z in zero_outs
    ]
    return concat_in + concat_zeros


def kernel(X, train_X, dual_coef):
    X = np.asarray(X, dtype=np.float32)
    train_X = np.asarray(train_X, dtype=np.float32)
    dual_coef = np.asarray(dual_coef, dtype=np.float32)

    fn, in_names, out_names, out_avals, zero_outs, mesh = _get_callable()
    in_maps = make_in_maps(X, train_X, dual_coef)
    args = concat_inputs(in_maps)
    outs = fn(*args)
    out = np.asarray(outs[0]).reshape(N_CORES, P, NQP)
    # device stores out[core, p, qp] for query core*QPC + qp*128 + p
    out = np.transpose(out, (0, 2, 1)).reshape(-1)
    return out.astype(np.float32)


# revision 18
# speedup vs baseline: 1.0503x; 1.0503x over previous
"""RBF kernel ridge regression inference on 8 Trainium2 NeuronCores — v3.

out[q] = sum_t exp(-g*||X[q]-T[t]||^2) * coef[t]
       = sum_t exp(2g*dot[q,t] - g*x2[q]) * w[t],   w[t] = coef[t]*exp(-g*y2[t])

Layout: queries on PSUM partitions, train on the free dim.  The GEMM runs in
fp8e4 DoubleRow (2 MACs/PE/cycle) with the query operand stationary; matmuls
are ordered dc-outer so each stationary weight is reused across the 4 psum
sections (2 LDWEIGHTS per tile instead of 8).  ScalarE evaluates one fused exp
per [128 x 2048] PSUM span with per-partition bias -g*x2[q] (precomputed on
the host, like w) and scale 2g.  The coef-weighted reduction over train runs
as a single fused tensor_tensor_reduce per tile on DVE (custom uop:
out = et*w, accum_out = sum), which avoids the slower scalar_tensor_tensor
path.  Queries are sharded across the 8 cores; train_X and w are replicated.
"""

import numpy as np
import ml_dtypes

GAMMA = 1.0
N_QUERY, N_TRAIN, D = 8192, 8192, 512
N_CORES = 8
P = 128
QPC = N_QUERY // N_CORES   # 1024 queries per core
NQP = QPC // P             # 8 query chunks (psum partition dim)
NDC = 2                    # DoubleRow contraction chunks (K=256 each)
TSEC = 512                 # train cols per matmul (one psum bank)
TGRP = 2048                # train cols per psum tile (4 banks)
NTG = N_TRAIN // TGRP      # 4 tile groups per query chunk
NSEC = TGRP // TSEC        # 4 sections per tile
# tile 0 is processed in 512-col sub-tiles: its (cold-clock) matmuls double as
# the HAM warmup while ACT/DVE start ~7us earlier than with a full first tile
SUB0 = [512, 512, 512, 512]

_CACHE = {}


def _build_program():
    from contextlib import ExitStack

    import concourse.bass as bass
    import concourse.mybir as mybir
    import concourse.tile as tile
    from concourse import bacc

    f32 = mybir.dt.float32
    bf16 = mybir.dt.bfloat16
    fp8 = mybir.dt.float8e4
    AF = mybir.ActivationFunctionType
    MUL = mybir.AluOpType.mult
    ADD = mybir.AluOpType.add
    DR = mybir.MatmulPerfMode.DoubleRow

    nc = bacc.Bacc(
        "TRN2", target_bir_lowering=False, debug=False, num_devices=N_CORES
    )

    # host layout: [p, dc, j, q/t] with d = dc*256 + j*128 + p
    xt_d = nc.dram_tensor("xt_fp8", [P, NDC * 2 * QPC], fp8, kind="ExternalInput").ap()
    tt_d = nc.dram_tensor("tt_fp8", [P, NDC * 2 * N_TRAIN], fp8, kind="ExternalInput").ap()
    w_d = nc.dram_tensor("w_bf16", [P, N_TRAIN], bf16, kind="ExternalInput").ap()
    x2_d = nc.dram_tensor("x2_f32", [P, NQP], f32, kind="ExternalInput").ap()
    out_d = nc.dram_tensor("out", [QPC], f32, kind="ExternalOutput").ap()

    with tile.TileContext(nc) as tc, ExitStack() as ctx:
        res = ctx.enter_context(tc.tile_pool(name="res", bufs=1))
        epool = ctx.enter_context(tc.tile_pool(name="ep", bufs=6))
        scrpool = ctx.enter_context(tc.tile_pool(name="scr", bufs=2))
        pspool = ctx.enter_context(tc.tile_pool(name="ps", bufs=2, space="PSUM"))

        # warm the exp table while input DMAs run
        warm = res.tile([1, 1], f32, tag="warm", name="warm")
        nc.vector.memset(warm[:], 0.0)
        nc.scalar.activation(warm[:], warm[:], AF.Exp)

        # ---- DMAs ordered by first use: tile 0 (qp0, first 512 train cols)
        # needs only ~450KB, so it starts computing ~2us in ----
        x2n = res.tile([P, NQP], f32, tag="x2n")
        xt_sb = res.tile([P, NDC, 2, QPC], fp8, tag="xt")
        w_sb = res.tile([P, N_TRAIN], bf16, tag="w")
        tt_sb = [
            res.tile([P, 2, N_TRAIN], fp8, tag=f"tt{dc}", name=f"tt{dc}")
            for dc in range(NDC)
        ]
        tt_view = tt_d.rearrange("p (dc j t) -> p dc j t", dc=NDC, j=2)
        xt_view = xt_d.rearrange("p (dc j q) -> p dc j q", dc=NDC, j=2)
        nc.sync.dma_start(x2n[:], x2_d)
        nc.sync.dma_start(xt_sb[:, :, :, :P], xt_view[:, :, :, :P])
        for dc in range(NDC):
            nc.sync.dma_start(tt_sb[dc][:, :, :TSEC], tt_view[:, dc, :, :TSEC])
        nc.sync.dma_start(w_sb[:, :TSEC], w_d[:, :TSEC])
        for dc in range(NDC):
            nc.sync.dma_start(
                tt_sb[dc][:, :, TSEC:TGRP], tt_view[:, dc, :, TSEC:TGRP]
            )
        nc.sync.dma_start(w_sb[:, TSEC:TGRP], w_d[:, TSEC:TGRP])
        nc.sync.dma_start(xt_sb[:, :, :, P:], xt_view[:, :, :, P:])
        for tg in range(1, NTG):
            t0, t1 = tg * TGRP, (tg + 1) * TGRP
            for dc in range(NDC):
                nc.sync.dma_start(tt_sb[dc][:, :, t0:t1], tt_view[:, dc, :, t0:t1])
            nc.sync.dma_start(w_sb[:, t0:t1], w_d[:, t0:t1])

        # ---- main loop (train-group major: consume tt as it streams in) ----
        # accum slots grouped contiguously per qp (tile 0 = one per sub-tile)
        counts = [NTG + len(SUB0) - 1] + [NTG] * (NQP - 1)
        bases = [sum(counts[:q]) for q in range(NQP)]
        next_slot = list(bases)
        Spart = res.tile([P, sum(counts)], f32, tag="Spart")
        for tg in range(NTG):
            for qp in range(NQP):
                subs = SUB0 if (tg == 0 and qp == 0) else [TGRP]
                ps = pspool.tile([P, TGRP], f32, tag="ps")
                et = epool.tile([P, TGRP], bf16, tag="exp")
                off = 0
                for wsub in subs:
                    for dc in range(NDC):
                        for sec in range(wsub // TSEC):
                            c0 = off + sec * TSEC
                            t0 = tg * TGRP + c0
                            nc.tensor.matmul(
                                ps[:, c0 : c0 + TSEC],
                                xt_sb[:, dc, :, qp * P : (qp + 1) * P],
                                tt_sb[dc][:, :, t0 : t0 + TSEC],
                                start=(dc == 0),
                                stop=(dc == NDC - 1),
                                perf_mode=DR,
                            )
                    nc.scalar.activation(
                        et[:, off : off + wsub], ps[:, off : off + wsub],
                        AF.Exp, bias=x2n[:, qp : qp + 1], scale=2.0 * GAMMA,
                    )
                    scr = scrpool.tile([P, TGRP], bf16, tag="rscr")
                    s = next_slot[qp]
                    next_slot[qp] += 1
                    nc.vector.scalar_tensor_tensor(
                        scr[:, :wsub], et[:, off : off + wsub], 1.0,
                        w_sb[:, tg * TGRP + off : tg * TGRP + off + wsub],
                        MUL, MUL, accum_out=Spart[:, s : s + 1],
                    )
                    off += wsub

        # ---- epilogue: out[qp] = sum over the qp's (contiguous) accum slots ----
        # out dram holds out[p*NQP + qp] = result for query qp*128+p: each
        # partition writes one contiguous 32B run (fast DMA); the host
        # transposes back (pure unshard gather).
        outcol = res.tile([P, NQP], f32, tag="outcol")
        for qp in range(NQP):
            nc.vector.tensor_reduce(
                outcol[:, qp : qp + 1],
                Spart[:, bases[qp] : bases[qp] + counts[qp]],
                axis=mybir.AxisListType.X,
                op=ADD,
            )
        nc.sync.dma_start(out_d.rearrange("(p c) -> p c", c=NQP), outcol[:])

    nc.compile()
    return nc


def _get_program():
    if "nc" not in _CACHE:
        _CACHE["nc"] = _build_program()
    return _CACHE["nc"]


def _interleave_fp8(mat_t):
    """[D, N] f32 -> [P, NDC*2*N] fp8 with host layout (p, dc, j, n),
    d = dc*256 + j*128 + p."""
    D_, N = mat_t.shape
    v = mat_t.reshape(NDC, 2, P, N)            # [dc, j, p, n]
    v = np.transpose(v, (2, 0, 1, 3))          # [p, dc, j, n]
    return np.ascontiguousarray(v.reshape(P, NDC * 2 * N)).astype(
        ml_dtypes.float8_e4m3
    )


def make_in_maps(X, train_X, dual_coef):
    bf = ml_dtypes.bfloat16
    # w[t] = coef[t] * exp(-g*||T[t]||^2): model-side constant, replicated.
    y2 = np.sum(train_X.astype(np.float64) ** 2, axis=1)
    w = (dual_coef.astype(np.float64) * np.exp(-GAMMA * y2)).astype(np.float32)
    ttb = _interleave_fp8(np.ascontiguousarray(train_X.T))
    wrep = np.ascontiguousarray(np.broadcast_to(w[None, :], (P, N_TRAIN))).astype(bf)
    # query-side bias -g*||X[q]||^2, laid out [p, qp] per core
    x2 = -GAMMA * np.sum(X.astype(np.float64) ** 2, axis=1)
    XT = np.ascontiguousarray(X.T)
    in_maps = []
    for c in range(N_CORES):
        xs = XT[:, c * QPC : (c + 1) * QPC]
        x2c = x2[c * QPC : (c + 1) * QPC].reshape(NQP, P).T  # [p, qp]
        in_maps.append(
            {
                "xt_fp8": _interleave_fp8(np.ascontiguousarray(xs)),
                "tt_fp8": ttb,
                "w_bf16": wrep,
                "x2_f32": np.ascontiguousarray(x2c).astype(np.float32),
            }
        )
    return in_maps


def _get_callable():
    """Cached jax shard_map callable for the 8-core NEFF execution."""
    if "call" in _CACHE:
        return _CACHE["call"]

    import jax
    from jax.sharding import Mesh, PartitionSpec
    from jax.experimental.shard_map import shard_map

    import concourse.mybir as mybir
    from concourse import bass2jax
    from concourse.bass2jax import install_neuronx_cc_hook

    install_neuronx_cc_hook()
    nc = _get_program()

    partition_name = (
        nc.partition_id_tensor.name if nc.partition_id_tensor else None
    )
    in_names, out_names, out_avals, zero_outs = [], [], [], []
    for alloc in nc.m.functions[0].allocations:
        if not isinstance(alloc, mybir.MemoryLocationSet):
            continue
        if alloc.kind not in ("ExternalInput", "ExternalOutput"):
            continue
        name = alloc.memorylocations[0].name
        if alloc.kind == "ExternalInput":
            if name != partition_name:
                in_names.append(name)
        else:
            out_names.append(name)
            shape = tuple(alloc.tensor_shape)
            dtype = mybir.dt.np(alloc.dtype)
            out_avals.append(jax.core.ShapedArray(shape, dtype))
            zero_outs.append(np.zeros(shape, dtype))
    all_in_names = in_names + out_names
    if partition_name is not None:
        all_in_names = all_in_names + [partition_name]

    def _body(*args):
        operands = list(args)
        if partition_name is not None:
            operands.append(bass2jax.partition_id_tensor())
        outs = bass2jax._bass_exec_p.bind(
            *operands,
            out_avals=tuple(out_avals),
            in_names=tuple(all_in_names),
            out_names=tuple(out_names),
            lowering_input_output_aliases=(),
            sim_require_finite=True,
            sim_require_nnan=True,
            nc=nc,
        )
        return tuple(outs)

    devices = jax.devices()[:N_CORES]
    mesh = Mesh(np.asarray(devices), ("core",))
    n_all = len(in_names) + len(out_names)
    fn = jax.jit(
        shard_map(
            _body,
            mesh=mesh,
            in_specs=(PartitionSpec("core"),) * n_all,
            out_specs=(PartitionSpec("core"),) * len(out_names),
            check_rep=False,
        ),
        keep_unused=True,
    )
    _CACHE["call"] = (fn, in_names, out_names, out_avals, zero_outs, mesh)
    return _CACHE["call"]


def concat_inputs(in_maps):
    fn, in_names, out_names, out_avals, zero_outs, mesh = _get_callable()
    concat_in = [
        np.concatenate([np.asarray(m[name]) for m in in_maps], axis=0)
        for name in in_names
    ]
    concat_zeros = [
        np.zeros((N_CORES * z.shape[0], *z.shape[1:]), z.dtype) for z in zero_outs
    ]
    return concat_in + concat_zeros


def kernel(X, train_X, dual_coef):
    X = np.asarray(X, dtype=np.float32)
    train_X = np.asarray(train_X, dtype=np.float32)
    dual_coef = np.asarray(dual_coef, dtype=np.float32)

    fn, in_names, out_names, out_avals, zero_outs, mesh = _get_callable()
    in_maps = make_in_maps(X, train_X, dual_coef)
    args = concat_inputs(in_maps)
    outs = fn(*args)
    out = np.asarray(outs[0]).reshape(N_CORES, P, NQP)
    # device stores out[core, p, qp] for query core*QPC + qp*128 + p
    out = np.transpose(out, (0, 2, 1)).reshape(-1)
    return out.astype(np.float32)
